# revision 1
# baseline (speedup 1.0000x reference)
"""3-layer GCN (DGL GraphConv, norm='both', ReLU) on 8 Trainium2 NeuronCores.

Strategy
--------
Nodes are sharded into 8 contiguous dst-ranges (6272 padded nodes per core);
each core owns all edges whose dst falls in its range.  Per 128-dst window the
edges are padded into 128-edge blocks (split by src half so gather indices fit
int16).  Per layer and per core:

  1. dma_gather (SWDGE, 4 queues) fetches x[src] rows (512 B each) for the
     core's edge slots, edge-major: partition = edge-slot-in-block.
  2. DVE builds a one-hot "scatter matrix" per block with a broadcast
     is_equal against an iota row (dst-local position of each edge).
  3. PE contracts edges:  psum[feat, dst] += E_block.T-free @ OneHot_block,
     accumulating all blocks of a window in one PSUM tile.
  4. Window epilogue: x indeg^-1/2 (replicated row vector), W matmul
     (stationary weights), bias+relu in one tensor_scalar, PE transpose back
     to node-major, x outdeg^-1/2 (pre-scale for the next layer's gather).
  5. AllGather of the computed shard -> full node array for the next layer.

All graph normalization is folded into per-node scales: the input h is
pre-scaled by outdeg^-1/2 on the host, and each layer's output is pre-scaled
for the next layer's gather.  When biases are zero, relu commutes with the
positive per-node scales, which is what makes the folding exact; nonzero
biases are handled by applying indeg^-1/2 before bias+relu (step 4), which is
exact regardless.
"""
import sys

sys.path.insert(0, "/opt/trn_rl_repo")

import numpy as np

N_CORES = 8
N_NODES = 50000
D = 128
NPAD = 50176          # 8 * 6272
SHARD = NPAD // N_CORES      # 6272 nodes per core
NWIN = SHARD // 128          # 49 dst windows per core
HALF = NPAD // 2             # 25088 (int16-safe gather base split)
BLOCKS_PER_CALL = 8          # 1024 idxs per dma_gather (SWDGE ring limit)
NQ = 4                       # SWDGE queues


def _set_geometry(n_nodes, npad, n_cores=8):
    """Test hook: shrink the problem (must keep npad % (n_cores*128) == 0)."""
    global N_NODES, NPAD, SHARD, NWIN, HALF, N_CORES
    N_CORES = n_cores
    N_NODES, NPAD = n_nodes, npad
    SHARD = NPAD // N_CORES
    NWIN = SHARD // 128
    HALF = NPAD // 2


# ----------------------------------------------------------------- host prep
def preprocess(h, src, dst, W0, b0, W1, b1, W2, b2):
    src = np.asarray(src).astype(np.int64)
    dst = np.asarray(dst).astype(np.int64)
    h = np.asarray(h, dtype=np.float32)

    out_deg = np.clip(np.bincount(src, minlength=N_NODES), 1, None).astype(np.float32)
    in_deg = np.clip(np.bincount(dst, minlength=N_NODES), 1, None).astype(np.float32)
    osc = out_deg ** -0.5                     # source-side scale
    isc = in_deg ** -0.5                      # dst-side scale

    # Degree-balanced serpentine renumbering: equalize per-window in-degree so
    # the cross-window block padding (KA/KB) stays near the mean.
    nwin_g = NPAD // 128
    ind_pad = np.zeros(NPAD, np.int64)
    ind_pad[:N_NODES] = np.bincount(dst, minlength=N_NODES)
    order = np.argsort(-ind_pad, kind="stable")
    perm = np.empty(NPAD, np.int64)           # new_id -> old_id
    for r in range(128):
        row = order[r * nwin_g:(r + 1) * nwin_g]
        if r % 2:
            row = row[::-1]
        perm[np.arange(nwin_g) * 128 + r] = row
    inv = np.empty(NPAD, np.int64)            # old_id -> new_id
    inv[perm] = np.arange(NPAD)
    src = inv[src]
    dst = inv[dst]

    osc_full = np.zeros(NPAD, np.float32)
    osc_full[:N_NODES] = osc
    isc_full = np.zeros(NPAD, np.float32)
    isc_full[:N_NODES] = isc
    osc_pad = osc_full[perm]                  # new-id order
    isc_pad = isc_full[perm]

    x0 = np.zeros((NPAD, D), np.float32)
    real = perm < N_NODES
    x0[real] = h[perm[real]] * osc[perm[real], None]

    meta_perm = perm
    # bucket edges: core -> window -> half -> list of (src, dst_local)
    core = dst // SHARD
    win = (dst % SHARD) // 128
    dloc = dst % 128
    half = (src >= HALF).astype(np.int64)
    # sort by (core, win, half) for grouped extraction
    order = np.lexsort((src, half, win, core))
    core_s, win_s, half_s, src_s, dloc_s = (
        core[order], win[order], half[order], src[order], dloc[order])

    # per (core, win, half) counts
    key = (core_s * NWIN + win_s) * 2 + half_s
    counts = np.bincount(key, minlength=N_CORES * NWIN * 2).reshape(N_CORES, NWIN, 2)
    nblk = -(-counts // 128)                  # ceil blocks needed
    KA = nblk[:, :, 0].max(axis=0)            # per-window A blocks (cross-core max)
    KB = nblk[:, :, 1].max(axis=0)
    KB = np.maximum(KB, (KA + KB == 0).astype(np.int64))   # >=1 block per window
    NA_BLK, NB_BLK = int(KA.sum()), int(KB.sum())

    # slot arrays per core: idx (half-local source row) + dst-local (or -1 pad)
    idxA = np.zeros((N_CORES, NA_BLK * 128), np.int16)
    idxB = np.zeros((N_CORES, NB_BLK * 128), np.int16)
    dlA = np.full((N_CORES, NA_BLK * 128), -1.0, np.float32)
    dlB = np.full((N_CORES, NB_BLK * 128), -1.0, np.float32)

    # start offset of each (core,win,half) run inside the sorted arrays
    run_starts = np.zeros(N_CORES * NWIN * 2 + 1, np.int64)
    np.cumsum(np.bincount(key, minlength=N_CORES * NWIN * 2), out=run_starts[1:])
    offA = np.concatenate(([0], np.cumsum(KA)))[:-1] * 128   # slot offset per window
    offB = np.concatenate(([0], np.cumsum(KB)))[:-1] * 128
    for c in range(N_CORES):
        for w in range(NWIN):
            for hf, (idxT, dlT, off) in (
                    (0, (idxA, dlA, offA)), (1, (idxB, dlB, offB))):
                k = (c * NWIN + w) * 2 + hf
                s, e = run_starts[k], run_starts[k + 1]
                n = e - s
                o = off[w]
                idxT[c, o:o + n] = (src_s[s:e] - hf * HALF).astype(np.int16)
                dlT[c, o:o + n] = dloc_s[s:e].astype(np.float32)

    def wrap_idx(a):
        # idx i -> [i % 16, i // 16], replicated to 128 partitions
        return np.tile(a.reshape(-1, 16).T, (8, 1)).copy()

    meta = {
        "KA": KA.astype(np.int64), "KB": KB.astype(np.int64),
        "NA_BLK": NA_BLK, "NB_BLK": NB_BLK, "perm": meta_perm,
    }
    per_core = []
    for c in range(N_CORES):
        sh = slice(c * SHARD, (c + 1) * SHARD)
        ins = {
            "x0": x0,
            "idxA": wrap_idx(idxA[c]), "idxB": wrap_idx(idxB[c]),
            "dlA": np.tile(dlA[c].reshape(-1, 128).T, 1).copy(),   # [128, NA_BLK]
            "dlB": np.tile(dlB[c].reshape(-1, 128).T, 1).copy(),
            "indeg_rep": np.tile(isc_pad[sh][None, :], (128, 1)).copy(),
            "outdeg_sc": osc_pad[sh].reshape(NWIN, 128).T.copy(),  # [128, NWIN]
            "W0": np.asarray(W0, np.float32), "W1": np.asarray(W1, np.float32),
            "W2": np.asarray(W2, np.float32),
            "b0": np.asarray(b0, np.float32).reshape(128, 1),
            "b1": np.asarray(b1, np.float32).reshape(128, 1),
            "b2": np.asarray(b2, np.float32).reshape(128, 1),
            "iota_rep": np.tile(np.arange(128, dtype=np.float32)[None, :], (128, 1)),
            "ident": np.eye(128, dtype=np.float32),
        }
        per_core.append(ins)
    return per_core, meta


# ------------------------------------------------------------- device program
def build_program(meta, repeat=1, collectives=True, ablate=None):
    import concourse.bacc as bacc
    import concourse.mybir as mybir
    from concourse.tile import TileContext, add_dep_helper
    from concourse import library_config

    KA, KB = meta["KA"], meta["KB"]
    NA_BLK, NB_BLK = meta["NA_BLK"], meta["NB_BLK"]
    f32 = mybir.dt.float32

    nc = bacc.Bacc("TRN2", target_bir_lowering=False, debug=False,
                   num_devices=N_CORES, num_swdge_queues=NQ)

    x0 = nc.dram_tensor("x0", [NPAD, D], f32, kind="ExternalInput")
    idxA = nc.dram_tensor("idxA", [128, NA_BLK * 8], mybir.dt.int16, kind="ExternalInput")
    idxB = nc.dram_tensor("idxB", [128, NB_BLK * 8], mybir.dt.int16, kind="ExternalInput")
    dlA = nc.dram_tensor("dlA", [128, NA_BLK], f32, kind="ExternalInput")
    dlB = nc.dram_tensor("dlB", [128, NB_BLK], f32, kind="ExternalInput")
    indeg_rep = nc.dram_tensor("indeg_rep", [128, SHARD], f32, kind="ExternalInput")
    outdeg_sc = nc.dram_tensor("outdeg_sc", [128, NWIN], f32, kind="ExternalInput")
    Ws = [nc.dram_tensor(f"W{i}", [128, 128], f32, kind="ExternalInput") for i in range(3)]
    bs = [nc.dram_tensor(f"b{i}", [128, 1], f32, kind="ExternalInput") for i in range(3)]
    iota_rep = nc.dram_tensor("iota_rep", [128, 128], f32, kind="ExternalInput")
    ident = nc.dram_tensor("ident", [128, 128], f32, kind="ExternalInput")
    out = nc.dram_tensor("out", [SHARD, D], f32, kind="ExternalOutput")

    # static call layout per stream: list of (start_block, nblocks)
    def calls_of(nblk_total):
        c, s = [], 0
        while s < nblk_total:
            n = min(BLOCKS_PER_CALL, nblk_total - s)
            c.append((s, n))
            s += n
        return c
    callsA, callsB = calls_of(NA_BLK), calls_of(NB_BLK)

    with TileContext(nc) as tc:
        with tc.tile_pool(name="const", bufs=1) as constp, \
             tc.tile_pool(name="idxp", bufs=1) as idxp, \
             tc.tile_pool(name="gatA", bufs=4) as gpa, \
             tc.tile_pool(name="gatB", bufs=4) as gpb, \
             tc.tile_pool(name="oh", bufs=3) as ohp, \
             tc.tile_pool(name="ep", bufs=3) as epp, \
             tc.tile_pool(name="pmm", bufs=3, space="PSUM") as pmm, \
             tc.tile_pool(name="pw", bufs=2, space="PSUM") as pw, \
             tc.tile_pool(name="pt", bufs=2, space="PSUM") as ptp, \
             tc.tile_pool(name="dram", bufs=1, space="DRAM") as dram:

            lib = nc.gpsimd.load_library(library_config.mlp)

            # resident constants
            itA = idxp.tile([128, NA_BLK * 8], mybir.dt.int16, name="itA")
            itB = idxp.tile([128, NB_BLK * 8], mybir.dt.int16, name="itB")
            dtA = idxp.tile([128, NA_BLK], f32, name="dtA")
            dtB = idxp.tile([128, NB_BLK], f32, name="dtB")
            nc.sync.dma_start(out=itA[:, :], in_=idxA[:, :])
            nc.sync.dma_start(out=itB[:, :], in_=idxB[:, :])
            nc.sync.dma_start(out=dtA[:, :], in_=dlA[:, :])
            nc.sync.dma_start(out=dtB[:, :], in_=dlB[:, :])
            ind_t = constp.tile([128, SHARD], f32, name="ind_t")
            nc.sync.dma_start(out=ind_t[:, :], in_=indeg_rep[:, :])
            osc_t = constp.tile([128, NWIN], f32, name="osc_t")
            nc.sync.dma_start(out=osc_t[:, :], in_=outdeg_sc[:, :])
            W_t = [constp.tile([128, 128], f32, name=f"W_t{i}") for i in range(3)]
            b_t = [constp.tile([128, 1], f32, name=f"b_t{i}") for i in range(3)]
            for i in range(3):
                nc.sync.dma_start(out=W_t[i][:, :], in_=Ws[i][:, :])
                nc.sync.dma_start(out=b_t[i][:, :], in_=bs[i][:, :])
            iota_t = constp.tile([128, 128], f32, name="iota_t")
            nc.sync.dma_start(out=iota_t[:, :], in_=iota_rep[:, :])
            id_t = constp.tile([128, 128], f32, name="id_t")
            nc.sync.dma_start(out=id_t[:, :], in_=ident[:, :])

            hfull = [dram.tile([NPAD, D], f32, name=f"hfull{i}") for i in range(2)]
            staging = [dram.tile([SHARD, D], f32, name=f"staging{i}") for i in range(2)]

            if repeat > 1:
                # seed hfull so no-collective timing variants read finite data
                for i in range(2):
                    nc.sync.dma_start(out=hfull[i][:, :], in_=x0[:, :])
            from contextlib import nullcontext
            swdge_k = [0]
            prev_g = [None]
            loop_cm = tc.For_i(0, repeat, 1) if repeat > 1 else nullcontext()
            with loop_cm:
              for layer in range(3):
                src_dram = x0 if layer == 0 else hfull[(layer - 1) % 2]

                # just-in-time per-call emission (consumption order) so the
                # fixed-size pools always admit a valid schedule.  All calls
                # share ONE tile tag so slot rotation == emission order ==
                # queue cycle; that keeps Tile's SWDGE sem-lane round-robin
                # consistent with queue_num.
                def emit_call(ci, calls, it, dt_tile, base_lo, sname, store):
                    s, n = calls[ci]
                    g = oh = None
                    if ablate != "oh_only":
                        g = gpa.tile([128, BLOCKS_PER_CALL * D], f32, tag="g",
                                     name=f"g{sname}_{layer}_{ci}")
                        g3 = g[:, :n * D].rearrange("p (b e) -> p b e", e=D)
                        gi = nc.gpsimd.dma_gather(
                            out_ap=g3,
                            in_ap=src_dram[base_lo:base_lo + HALF, :],
                            idxs_ap=it[:, s * 8:(s * 8 + n * 8)],
                            num_idxs=n * 128, num_idxs_reg=n * 128,
                            elem_size=D, queue_num=swdge_k[0] % NQ)
                        swdge_k[0] += 1
                        add_dep_helper(lib.ins, gi.ins, True, "lib first")
                    if ablate != "gather_only":
                        oh = ohp.tile([128, BLOCKS_PER_CALL * 128], f32, tag="oh",
                                      name=f"oh{sname}_{layer}_{ci}")
                        in0 = iota_t[:, :].unsqueeze(1).broadcast_to([128, n, 128])
                        in1 = dt_tile[:, s:s + n].unsqueeze(2).broadcast_to([128, n, 128])
                        nc.vector.tensor_tensor(
                            oh[:, :n * 128].rearrange("p (b e) -> p b e", e=128),
                            in0, in1, mybir.AluOpType.is_equal)
                    for j in range(n):
                        store[s + j] = (g, j, oh, j)

                tilesA, tilesB = {}, {}
                eA = eB = 0

                W_l, b_l = W_t[layer], b_t[layer]
                aoff = boff = 0
                for w in range(NWIN):
                    ka, kb = int(KA[w]), int(KB[w])
                    need_a = -(-(aoff + ka) // BLOCKS_PER_CALL)
                    need_b = -(-(boff + kb) // BLOCKS_PER_CALL)
                    while eA < min(need_a, len(callsA)):
                        emit_call(eA, callsA, itA, dtA, 0, "A", tilesA)
                        eA += 1
                    while eB < min(need_b, len(callsB)):
                        emit_call(eB, callsB, itB, dtB, HALF, "B", tilesB)
                        eB += 1
                    if ablate in ("gather", "gather_only", "oh_only"):
                        aoff += ka
                        boff += kb
                        continue
                    ntot = ka + kb
                    pm = pmm.tile([128, 128], f32, tag="pm", name=f"pm_{layer}_{w}")
                    bi = 0
                    for hf, cnt, off, tiles in (
                            (0, ka, aoff, tilesA), (1, kb, boff, tilesB)):
                        for j in range(cnt):
                            g, gj, oh, oj = tiles[off + j]
                            nc.tensor.matmul(
                                pm[:, :],
                                g[:, gj * D:(gj + 1) * D],
                                oh[:, oj * 128:(oj + 1) * 128],
                                start=(bi == 0), stop=(bi == ntot - 1))
                            bi += 1
                    aoff += ka
                    boff += kb

                    # epilogue
                    mt = epp.tile([128, 128], f32, tag="mt", name=f"mt_{layer}_{w}")
                    nc.vector.tensor_tensor(
                        mt[:, :], pm[:, :], ind_t[:, w * 128:(w + 1) * 128],
                        mybir.AluOpType.mult)
                    p2 = pw.tile([128, 128], f32, tag="p2", name=f"p2_{layer}_{w}")
                    nc.tensor.matmul(p2[:, :], W_l[:, :], mt[:, :], start=True, stop=True)
                    hT = epp.tile([128, 128], f32, tag="hT", name=f"hT_{layer}_{w}")
                    nc.vector.tensor_scalar(
                        hT[:, :], p2[:, :], b_l[:, :1], 0.0,
                        mybir.AluOpType.add, mybir.AluOpType.max)
                    p3 = ptp.tile([128, 128], f32, tag="p3", name=f"p3_{layer}_{w}")
                    nc.tensor.transpose(p3[:, :], hT[:, :], id_t[:, :])
                    hn = epp.tile([128, 128], f32, tag="hn", name=f"hn_{layer}_{w}")
                    if layer < 2:
                        nc.vector.tensor_scalar(
                            hn[:, :], p3[:, :], osc_t[:, w:w + 1], None,
                            mybir.AluOpType.mult)
                        nc.sync.dma_start(
                            out=staging[layer][w * 128:(w + 1) * 128, :], in_=hn[:, :])
                    else:
                        nc.vector.tensor_copy(hn[:, :], p3[:, :])
                        nc.sync.dma_start(
                            out=out[w * 128:(w + 1) * 128, :], in_=hn[:, :])

                if layer < 2 and collectives:
                    nc.gpsimd.collective_compute(
                        "AllGather", mybir.AluOpType.bypass,
                        replica_groups=[list(range(N_CORES))],
                        ins=[staging[layer][:, :].opt()],
                        outs=[hfull[layer % 2][:, :].opt()],
                    )

    nc.compile()
    return nc


_CACHE = {}


def _get_program(meta):
    key = (tuple(meta["KA"]), tuple(meta["KB"]))
    if key not in _CACHE:
        _CACHE[key] = build_program(meta)
    return _CACHE[key]


def kernel(h, src, dst, W0, b0, W1, b1, W2, b2):
    from concourse.bass_utils import run_bass_kernel_spmd

    per_core, meta = preprocess(h, src, dst, W0, b0, W1, b1, W2, b2)
    nc = _get_program(meta)
    res = run_bass_kernel_spmd(nc, per_core, core_ids=list(range(N_CORES)))
    shards = [res.results[c]["out"] for c in range(N_CORES)]
    full_new = np.concatenate(shards, axis=0)      # new-id order [NPAD, D]
    perm = meta["perm"]
    out = np.empty((N_NODES, D), np.float32)
    real = perm < N_NODES
    out[perm[real]] = full_new[real]
    return out



# revision 13
# speedup vs baseline: 142.6495x; 142.6495x over previous
"""3-layer GCN (DGL GraphConv, norm='both', ReLU) on 8 Trainium2 NeuronCores.

Strategy
--------
Nodes are renumbered (degree-balanced serpentine) into 392 balanced windows
of 128 dst nodes; core c owns windows [49c, 49c+49) so a single AllGather
concatenates rank shards in order.  Each core owns all edges whose dst falls
in its windows.  The feature pipeline runs in bf16 (tolerance is 2e-2; bf16
keeps ~0.4% rel error):

  1. dma_gather (SWDGE, 4 queues, 6 calls in flight per stream) fetches
     x[src] rows (256 B bf16) for the core's edge slots, edge-major:
     partition = edge-slot-in-block.
  2. DVE builds a one-hot "scatter matrix" per block with a broadcast
     is_equal against an iota row (dst-local position of each edge), bf16.
  3. PE contracts edges:  psum[feat, dst] += E_block.T-free @ OneHot_block,
     accumulating all blocks of a window in one PSUM tile (bf16 matmuls,
     fp32 PSUM).
  4. Window epilogue: x indeg^-1/2 (replicated row vector), W matmul
     (stationary bf16 weights), bias+relu in one tensor_scalar, PE transpose
     back to node-major, x outdeg^-1/2 (pre-scale for the next layer's
     gather), staging write (bf16).
  5. One AllGather per layer with a Shared-output DRAM tensor (the fast
     direct-write path; a Local output costs ~200x more) -> full node array
     for the next layer.

All graph normalization is folded into per-node scales: the input h is
pre-scaled by outdeg^-1/2 on the host, and each layer's output is pre-scaled
for the next layer's gather.  Biases are applied after indeg^-1/2 (step 4),
which keeps the folding exact even for nonzero b.
"""
import sys

sys.path.insert(0, "/opt/trn_rl_repo")

import numpy as np
import ml_dtypes

BF16 = ml_dtypes.bfloat16

N_CORES = 8
N_NODES = 50000
D = 128
NPAD = 50176          # 8 * 6272
SHARD = NPAD // N_CORES      # 6272 nodes per core
NWIN = SHARD // 128          # 49 dst windows per core
HALF = NPAD // 2             # 25088 (int16-safe gather base split)
BLOCKS_PER_CALL = 8          # 1024 idxs per dma_gather (SWDGE ring limit)
NQ = 4                       # SWDGE queues


def _core_win_of_g():
    """global window g -> (core, local window).  Contiguous per-core layout:
    a single AllGather concatenates rank shards in rank order."""
    g = np.arange(NPAD // 128)
    return g // NWIN, g % NWIN


def _g_of_core_win():
    """(core, local window) -> global window."""
    core_of_g, w_of_g = _core_win_of_g()
    g = np.empty((N_CORES, NWIN), np.int64)
    g[core_of_g, w_of_g] = np.arange(NPAD // 128)
    return g


def wrap_idx(a):
    # idx i -> [i % 16, i // 16], replicated to 128 partitions
    return np.tile(a.reshape(-1, 16).T, (8, 1)).copy()


# ----------------------------------------------------------------- host prep
def preprocess(h, src, dst, W0, b0, W1, b1, W2, b2):
    src = np.asarray(src).astype(np.int64)
    dst = np.asarray(dst).astype(np.int64)
    h = np.asarray(h, dtype=np.float32)

    out_deg = np.clip(np.bincount(src, minlength=N_NODES), 1, None).astype(np.float32)
    in_deg = np.clip(np.bincount(dst, minlength=N_NODES), 1, None).astype(np.float32)
    osc = out_deg ** -0.5                     # source-side scale
    isc = in_deg ** -0.5                      # dst-side scale

    # Degree-balanced serpentine renumbering: equalize per-window in-degree so
    # the cross-window block padding (KA/KB) stays near the mean.
    nwin_g = NPAD // 128
    ind_pad = np.zeros(NPAD, np.int64)
    ind_pad[:N_NODES] = np.bincount(dst, minlength=N_NODES)
    order = np.argsort(-ind_pad, kind="stable")
    perm = np.empty(NPAD, np.int64)           # new_id -> old_id
    for r in range(128):
        row = order[r * nwin_g:(r + 1) * nwin_g]
        if r % 2:
            row = row[::-1]
        perm[np.arange(nwin_g) * 128 + r] = row
    inv = np.empty(NPAD, np.int64)            # old_id -> new_id
    inv[perm] = np.arange(NPAD)
    src = inv[src]
    dst = inv[dst]

    osc_full = np.zeros(NPAD, np.float32)
    osc_full[:N_NODES] = osc
    isc_full = np.zeros(NPAD, np.float32)
    isc_full[:N_NODES] = isc
    osc_pad = osc_full[perm]                  # new-id order
    isc_pad = isc_full[perm]

    x0 = np.zeros((NPAD, D), np.float32)
    real = perm < N_NODES
    x0[real] = h[perm[real]] * osc[perm[real], None]
    x0 = x0.astype(BF16)

    core_of_g, w_of_g = _core_win_of_g()
    g_of_cw = _g_of_core_win()
    # node ids of core c in local-window order: [NWIN*128]
    node_ids = (g_of_cw[:, :, None] * 128 +
                np.arange(128)[None, None, :]).reshape(N_CORES, SHARD)

    # bucket edges: core -> window -> half -> list of (src, dst_local)
    gw = dst // 128
    core = core_of_g[gw]
    win = w_of_g[gw]
    dloc = dst % 128
    half = (src >= HALF).astype(np.int64)
    # sort by (core, win, half) for grouped extraction
    order = np.lexsort((src, half, win, core))
    core_s, win_s, half_s, src_s, dloc_s = (
        core[order], win[order], half[order], src[order], dloc[order])

    # per (core, win, half) counts
    key = (core_s * NWIN + win_s) * 2 + half_s
    counts = np.bincount(key, minlength=N_CORES * NWIN * 2).reshape(N_CORES, NWIN, 2)
    nblk = -(-counts // 128)                  # ceil blocks needed
    KA = nblk[:, :, 0].max(axis=0)            # per-window A blocks (cross-core max)
    KB = nblk[:, :, 1].max(axis=0)
    KB = np.maximum(KB, (KA + KB == 0).astype(np.int64))   # >=1 block per window
    NA_BLK, NB_BLK = int(KA.sum()), int(KB.sum())

    # slot arrays per core: idx (half-local source row) + dst-local (or -1 pad)
    idxA = np.zeros((N_CORES, NA_BLK * 128), np.int16)
    idxB = np.zeros((N_CORES, NB_BLK * 128), np.int16)
    dlA = np.full((N_CORES, NA_BLK * 128), -1.0, np.float32)
    dlB = np.full((N_CORES, NB_BLK * 128), -1.0, np.float32)

    # start offset of each (core,win,half) run inside the sorted arrays
    run_starts = np.zeros(N_CORES * NWIN * 2 + 1, np.int64)
    np.cumsum(np.bincount(key, minlength=N_CORES * NWIN * 2), out=run_starts[1:])
    offA = np.concatenate(([0], np.cumsum(KA)))[:-1] * 128   # slot offset per window
    offB = np.concatenate(([0], np.cumsum(KB)))[:-1] * 128
    for c in range(N_CORES):
        for w in range(NWIN):
            for hf, (idxT, dlT, off) in (
                    (0, (idxA, dlA, offA)), (1, (idxB, dlB, offB))):
                k = (c * NWIN + w) * 2 + hf
                s, e = run_starts[k], run_starts[k + 1]
                n = e - s
                o = off[w]
                idxT[c, o:o + n] = (src_s[s:e] - hf * HALF).astype(np.int16)
                dlT[c, o:o + n] = dloc_s[s:e].astype(np.float32)

    meta = {
        "KA": KA.astype(np.int64), "KB": KB.astype(np.int64),
        "NA_BLK": NA_BLK, "NB_BLK": NB_BLK, "perm": perm,
        "node_ids": node_ids,
    }
    per_core = []
    for c in range(N_CORES):
        ids = node_ids[c]
        ins = {
            "x0": x0,
            "idxA": wrap_idx(idxA[c]), "idxB": wrap_idx(idxB[c]),
            "dlA": np.tile(dlA[c].reshape(-1, 128).T, 1).astype(BF16).copy(),
            "dlB": np.tile(dlB[c].reshape(-1, 128).T, 1).astype(BF16).copy(),
            "indeg_rep": np.tile(isc_pad[ids][None, :], (128, 1)).copy(),
            "outdeg_sc": osc_pad[ids].reshape(NWIN, 128).T.copy(),  # [128, NWIN]
            "W0": np.asarray(W0, np.float32).astype(BF16),
            "W1": np.asarray(W1, np.float32).astype(BF16),
            "W2": np.asarray(W2, np.float32).astype(BF16),
            "b0": np.asarray(b0, np.float32).reshape(128, 1),
            "b1": np.asarray(b1, np.float32).reshape(128, 1),
            "b2": np.asarray(b2, np.float32).reshape(128, 1),
            "iota_rep": np.tile(np.arange(128, dtype=np.float32)[None, :],
                                (128, 1)).astype(BF16),
            "ident": np.eye(128, dtype=np.float32).astype(BF16),
        }
        per_core.append(ins)
    return per_core, meta


# ------------------------------------------------------------- device program
def build_program(meta, repeat=1, collectives=True, ablate=None):
    import concourse.bacc as bacc
    import concourse.mybir as mybir
    from concourse.tile import TileContext, add_dep_helper
    from concourse import library_config

    KA, KB = meta["KA"], meta["KB"]
    NA_BLK, NB_BLK = meta["NA_BLK"], meta["NB_BLK"]
    f32 = mybir.dt.float32
    bf16 = mybir.dt.bfloat16

    nc = bacc.Bacc("TRN2", target_bir_lowering=False, debug=False,
                   num_devices=N_CORES, num_swdge_queues=NQ)

    x0 = nc.dram_tensor("x0", [NPAD, D], bf16, kind="ExternalInput")
    idxA = nc.dram_tensor("idxA", [128, NA_BLK * 8], mybir.dt.int16, kind="ExternalInput")
    idxB = nc.dram_tensor("idxB", [128, NB_BLK * 8], mybir.dt.int16, kind="ExternalInput")
    dlA = nc.dram_tensor("dlA", [128, NA_BLK], bf16, kind="ExternalInput")
    dlB = nc.dram_tensor("dlB", [128, NB_BLK], bf16, kind="ExternalInput")
    indeg_rep = nc.dram_tensor("indeg_rep", [128, SHARD], f32, kind="ExternalInput")
    outdeg_sc = nc.dram_tensor("outdeg_sc", [128, NWIN], f32, kind="ExternalInput")
    Ws = [nc.dram_tensor(f"W{i}", [128, 128], bf16, kind="ExternalInput") for i in range(3)]
    bs = [nc.dram_tensor(f"b{i}", [128, 1], f32, kind="ExternalInput") for i in range(3)]
    iota_rep = nc.dram_tensor("iota_rep", [128, 128], bf16, kind="ExternalInput")
    ident = nc.dram_tensor("ident", [128, 128], bf16, kind="ExternalInput")
    out = nc.dram_tensor("out", [SHARD, D], f32, kind="ExternalOutput")

    # static call layout per stream: list of (start_block, nblocks)
    def calls_of(nblk_total):
        c, s = [], 0
        while s < nblk_total:
            n = min(BLOCKS_PER_CALL, nblk_total - s)
            c.append((s, n))
            s += n
        return c
    callsA, callsB = calls_of(NA_BLK), calls_of(NB_BLK)

    # collectives can't sit inside control flow (For_i) -> python-unroll them
    use_fori = repeat > 1 and not collectives
    reps = repeat if (collectives and repeat > 1) else 1

    with TileContext(nc) as tc:
        with tc.tile_pool(name="const", bufs=1) as constp, \
             tc.tile_pool(name="idxp", bufs=1) as idxp, \
             tc.tile_pool(name="gatA", bufs=6) as gpa, \
             tc.tile_pool(name="gatB", bufs=6) as gpb, \
             tc.tile_pool(name="oh", bufs=3) as ohp, \
             tc.tile_pool(name="ep", bufs=3) as epp, \
             tc.tile_pool(name="pmm", bufs=3, space="PSUM") as pmm, \
             tc.tile_pool(name="pw", bufs=2, space="PSUM") as pw, \
             tc.tile_pool(name="pt", bufs=2, space="PSUM") as ptp, \
             tc.tile_pool(name="dram", bufs=1, space="DRAM") as dram:

            lib = nc.gpsimd.load_library(library_config.mlp)

            # resident constants
            itA = idxp.tile([128, NA_BLK * 8], mybir.dt.int16, name="itA")
            itB = idxp.tile([128, NB_BLK * 8], mybir.dt.int16, name="itB")
            dtA = idxp.tile([128, NA_BLK], bf16, name="dtA")
            dtB = idxp.tile([128, NB_BLK], bf16, name="dtB")
            nc.sync.dma_start(out=itA[:, :], in_=idxA[:, :])
            nc.sync.dma_start(out=itB[:, :], in_=idxB[:, :])
            nc.sync.dma_start(out=dtA[:, :], in_=dlA[:, :])
            nc.sync.dma_start(out=dtB[:, :], in_=dlB[:, :])
            ind_t = constp.tile([128, SHARD], f32, name="ind_t")
            nc.sync.dma_start(out=ind_t[:, :], in_=indeg_rep[:, :])
            osc_t = constp.tile([128, NWIN], f32, name="osc_t")
            nc.sync.dma_start(out=osc_t[:, :], in_=outdeg_sc[:, :])
            W_t = [constp.tile([128, 128], bf16, name=f"W_t{i}") for i in range(3)]
            b_t = [constp.tile([128, 1], f32, name=f"b_t{i}") for i in range(3)]
            for i in range(3):
                nc.sync.dma_start(out=W_t[i][:, :], in_=Ws[i][:, :])
                nc.sync.dma_start(out=b_t[i][:, :], in_=bs[i][:, :])
            iota_t = constp.tile([128, 128], bf16, name="iota_t")
            nc.sync.dma_start(out=iota_t[:, :], in_=iota_rep[:, :])
            id_t = constp.tile([128, 128], bf16, name="id_t")
            nc.sync.dma_start(out=id_t[:, :], in_=ident[:, :])

            # Shared DRAM allows a single writer instruction per tensor, so the
            # unrolled-collective timing variants get one hfull pair per rep.
            n_hf = reps if reps > 1 else 1
            hfull_sets = [
                [dram.tile([NPAD, D], bf16, name=f"hfull{i}_{r}",
                           addr_space="Shared") for i in range(2)]
                for r in range(n_hf)
            ]
            staging = [dram.tile([SHARD, D], bf16, name=f"staging{i}") for i in range(2)]

            if use_fori:
                # seed hfull so no-collective timing variants read finite data
                for i in range(2):
                    nc.sync.dma_start(out=hfull_sets[0][i][:, :], in_=x0[:, :])
            from contextlib import nullcontext
            swdge_k = [0]
            loop_cm = tc.For_i(0, repeat, 1) if use_fori else nullcontext()
            with loop_cm:
              for rep in range(reps):
                hfull = hfull_sets[rep % n_hf]
                for layer in range(3):
                    src_dram = x0 if layer == 0 else hfull[(layer - 1) % 2]

                    # just-in-time per-call emission (consumption order) so the
                    # fixed-size pools always admit a valid schedule.  All calls
                    # share ONE tile tag so slot rotation == emission order ==
                    # queue cycle; that keeps Tile's SWDGE sem-lane round-robin
                    # consistent with queue_num.
                    def emit_call(ci, calls, it, dt_tile, base_lo, gp, sname, store):
                        s, n = calls[ci]
                        g = oh = None
                        if ablate != "oh_only":
                            g = gp.tile([128, BLOCKS_PER_CALL * D], bf16, tag="g",
                                        name=f"g{sname}_{rep}_{layer}_{ci}")
                            g3 = g[:, :n * D].rearrange("p (b e) -> p b e", e=D)
                            gi = nc.gpsimd.dma_gather(
                                out_ap=g3,
                                in_ap=src_dram[base_lo:base_lo + HALF, :],
                                idxs_ap=it[:, s * 8:(s * 8 + n * 8)],
                                num_idxs=n * 128, num_idxs_reg=n * 128,
                                elem_size=D, queue_num=swdge_k[0] % NQ)
                            swdge_k[0] += 1
                            add_dep_helper(lib.ins, gi.ins, True, "lib first")
                        if ablate != "gather_only":
                            oh = ohp.tile([128, BLOCKS_PER_CALL * 128], bf16, tag="oh",
                                          name=f"oh{sname}_{rep}_{layer}_{ci}")
                            in0 = iota_t[:, :].unsqueeze(1).broadcast_to([128, n, 128])
                            in1 = dt_tile[:, s:s + n].unsqueeze(2).broadcast_to([128, n, 128])
                            nc.vector.tensor_tensor(
                                oh[:, :n * 128].rearrange("p (b e) -> p b e", e=128),
                                in0, in1, mybir.AluOpType.is_equal)
                        for j in range(n):
                            store[s + j] = (g, j, oh, j)

                    tilesA, tilesB = {}, {}
                    eA = eB = 0

                    W_l, b_l = W_t[layer], b_t[layer]
                    aoff = boff = 0
                    for w in range(NWIN):
                        ka, kb = int(KA[w]), int(KB[w])
                        need_a = -(-(aoff + ka) // BLOCKS_PER_CALL)
                        need_b = -(-(boff + kb) // BLOCKS_PER_CALL)
                        while eA < min(need_a, len(callsA)):
                            emit_call(eA, callsA, itA, dtA, 0, gpa, "A", tilesA)
                            eA += 1
                        while eB < min(need_b, len(callsB)):
                            emit_call(eB, callsB, itB, dtB, HALF, gpb, "B", tilesB)
                            eB += 1
                        if ablate in ("gather", "gather_only", "oh_only"):
                            aoff += ka
                            boff += kb
                            continue
                        ntot = ka + kb
                        pm = pmm.tile([128, 128], f32, tag="pm", name=f"pm_{rep}_{layer}_{w}")
                        bi = 0
                        for hf, cnt, off, tiles in (
                                (0, ka, aoff, tilesA), (1, kb, boff, tilesB)):
                            for j in range(cnt):
                                g, gj, oh, oj = tiles[off + j]
                                nc.tensor.matmul(
                                    pm[:, :],
                                    g[:, gj * D:(gj + 1) * D],
                                    oh[:, oj * 128:(oj + 1) * 128],
                                    start=(bi == 0), stop=(bi == ntot - 1))
                                bi += 1
                        aoff += ka
                        boff += kb

                        # epilogue
                        mt = epp.tile([128, 128], bf16, tag="mt", name=f"mt_{rep}_{layer}_{w}")
                        nc.vector.tensor_tensor(
                            mt[:, :], pm[:, :], ind_t[:, w * 128:(w + 1) * 128],
                            mybir.AluOpType.mult)
                        p2 = pw.tile([128, 128], f32, tag="p2", name=f"p2_{rep}_{layer}_{w}")
                        nc.tensor.matmul(p2[:, :], W_l[:, :], mt[:, :], start=True, stop=True)
                        hT = epp.tile([128, 128], bf16, tag="hT", name=f"hT_{rep}_{layer}_{w}")
                        nc.vector.tensor_scalar(
                            hT[:, :], p2[:, :], b_l[:, :1], 0.0,
                            mybir.AluOpType.add, mybir.AluOpType.max)
                        p3 = ptp.tile([128, 128], bf16, tag="p3", name=f"p3_{rep}_{layer}_{w}")
                        nc.tensor.transpose(p3[:, :], hT[:, :], id_t[:, :])
                        hn = epp.tile([128, 128], bf16 if layer < 2 else f32,
                                      tag=f"hn{min(layer,1)}", name=f"hn_{rep}_{layer}_{w}")
                        if layer < 2:
                            nc.vector.tensor_scalar(
                                hn[:, :], p3[:, :], osc_t[:, w:w + 1], None,
                                mybir.AluOpType.mult)
                            nc.sync.dma_start(
                                out=staging[layer][w * 128:(w + 1) * 128, :], in_=hn[:, :])
                        else:
                            nc.vector.tensor_copy(hn[:, :], p3[:, :])
                            nc.sync.dma_start(
                                out=out[w * 128:(w + 1) * 128, :], in_=hn[:, :])

                    if layer < 2 and collectives:
                        nc.gpsimd.collective_compute(
                            "AllGather", mybir.AluOpType.bypass,
                            replica_groups=[list(range(N_CORES))],
                            ins=[staging[layer][:, :].opt()],
                            outs=[hfull[layer % 2][:, :].opt()],
                        )

    nc.compile()
    return nc


_CACHE = {}


def _get_program(meta):
    key = (tuple(meta["KA"]), tuple(meta["KB"]))
    if key not in _CACHE:
        _CACHE[key] = build_program(meta)
    return _CACHE[key]


def kernel(h, src, dst, W0, b0, W1, b1, W2, b2):
    from concourse.bass_utils import run_bass_kernel_spmd

    per_core, meta = preprocess(h, src, dst, W0, b0, W1, b1, W2, b2)
    nc = _get_program(meta)
    res = run_bass_kernel_spmd(nc, per_core, core_ids=list(range(N_CORES)))
    full_new = np.empty((NPAD, D), np.float32)
    for c in range(N_CORES):
        full_new[meta["node_ids"][c]] = res.results[c]["out"]
    perm = meta["perm"]
    out = np.empty((N_NODES, D), np.float32)
    real = perm < N_NODES
    out[perm[real]] = full_new[real]
    return out
